# revision 1
# baseline (speedup 1.0000x reference)
"""Distributed Trainium2 Bass kernel for AtnConv (contextual-attention conv).

Sharding: 8 cores = batch(2) x position-blocks(4). Each core owns 1024 of the
4096 output positions of one sample and attends over ALL L=4096 patches, so the
softmax over L is core-local (free-axis reduction, no collectives needed).

Device per core (bf16 matmuls, fp32 PSUM/softmax):
  scores S^T[pos,L] = X_shard^T @ A'   (A' = normalized patches * mask * SCALE)
  masked softmax over L (stable: block max + deferred rescale)
  U^T[pos, C*4*4]  = Y^T @ R'          (R' = raw 4x4 patches * mask)
Host: im2col prep, col2im scatter-add, clip correction, final 4 dilated convs.
"""

import numpy as np
import ml_dtypes

B, C, H1, H2 = 2, 128, 128, 64
L = H2 * H2            # 4096 patches
POSL = 1024            # positions per core
KDIM = 1152            # 128*3*3 contraction for scores
RDIM = 2048            # 128*4*4 conv-transpose channels
SCALE = 10.0
EPS_NORM = 1e-4
EPS_CLAMP = 1e-8
BF16 = ml_dtypes.bfloat16

_NC = None


def _build_nc():
    import concourse.bass as bass
    import concourse.bacc as bacc
    import concourse.mybir as mybir
    from concourse import tile

    bf = mybir.dt.bfloat16
    f32 = mybir.dt.float32
    Exp = mybir.ActivationFunctionType.Exp
    X = mybir.AxisListType.X

    nc = bacc.Bacc(None, target_bir_lowering=False)
    xT = nc.declare_dram_parameter("xT", [9, 128, POSL], bf, isOutput=False)
    aT = nc.declare_dram_parameter("aT", [9, 128, L], bf, isOutput=False)
    rT = nc.declare_dram_parameter("rT", [32, 128, RDIM], bf, isOutput=False)
    uT = nc.declare_dram_parameter("uT", [POSL, RDIM], f32, isOutput=True)

    with tile.TileContext(nc) as tc:
        with (
            tc.tile_pool(name="big", bufs=1) as big,
            tc.tile_pool(name="st", bufs=1) as st,
            tc.tile_pool(name="wk", bufs=2) as wk,
            tc.tile_pool(name="rp", bufs=1) as rp,
            tc.tile_pool(name="ps", bufs=2, space=bass.MemorySpace.PSUM) as ps,
        ):
            estore = big.tile([128, 8, L], bf)        # 64 KiB/part
            ybuf = big.tile([128, 32, 512], bf)       # 32 KiB/part
            xt = big.tile([128, 9, POSL], bf)         # 18 KiB/part
            nbmaxs = st.tile([128, 8, 8], f32)
            rss = st.tile([128, 8, 8], f32)

            for k in range(9):
                nc.sync.dma_start(xt[:, k, :], xT[k])

            # ---- score matmul + block-local exp ----
            for n in range(8):                        # L blocks of 512
                a_n = wk.tile([128, 9, 512], bf, tag="a_n")
                for k in range(9):
                    nc.sync.dma_start(a_n[:, k, :], aT[k][:, n * 512:(n + 1) * 512])
                for m in range(8):                    # pos tiles of 128
                    z = ps.tile([128, 512], f32, tag="z")
                    for k in range(9):
                        nc.tensor.matmul(
                            z[:], xt[:, k, m * 128:(m + 1) * 128], a_n[:, k, :],
                            start=(k == 0), stop=(k == 8))
                    # nbmaxs holds NEGATED block maxes (reduce negate=True)
                    nc.vector.reduce_max(nbmaxs[:, m, n:n + 1], z[:], axis=X,
                                         negate=True)
                    ef = wk.tile([128, 512], f32, tag="ef")
                    nc.scalar.activation(ef[:], z[:], Exp,
                                         bias=nbmaxs[:, m, n:n + 1], scale=1.0)
                    nc.vector.reduce_sum(rss[:, m, n:n + 1], ef[:], axis=X)
                    nc.vector.tensor_copy(estore[:, m, n * 512:(n + 1) * 512], ef[:])

            # ---- softmax finalize: rescale each block by exp(bmax-gmax)/Z ----
            for m in range(8):
                # ngm = min_n(-bmax_n) = -gmax
                ngm = wk.tile([128, 1], f32, tag="ngm")
                nc.vector.tensor_reduce(ngm[:], nbmaxs[:, m, :], axis=X,
                                        op=mybir.AluOpType.min)
                # al_n = exp(-(nbmax_n)*1 ... ) = exp(bmax_n - gmax)
                al = wk.tile([128, 8], f32, tag="al")
                nc.scalar.activation(al[:], nbmaxs[:, m, :], Exp, bias=ngm[:],
                                     scale=-1.0)
                pr = wk.tile([128, 8], f32, tag="pr")
                nc.vector.tensor_mul(pr[:], al[:], rss[:, m, :])
                sm = wk.tile([128, 1], f32, tag="sm")
                nc.vector.reduce_sum(sm[:], pr[:], axis=X)
                rc = wk.tile([128, 1], f32, tag="rc")
                nc.vector.reciprocal(rc[:], sm[:])
                be = wk.tile([128, 8], f32, tag="be")
                nc.vector.tensor_scalar_mul(be[:], al[:], rc[:])
                for n in range(8):
                    nc.vector.tensor_scalar_mul(
                        estore[:, m, n * 512:(n + 1) * 512],
                        estore[:, m, n * 512:(n + 1) * 512], be[:, n:n + 1])

            # ---- transpose Y^T -> Y, then U^T = Y^T @ R' ----
            for q in range(2):                        # pos halves of 512
                for mm in range(4):
                    for k in range(32):
                        nc.sync.dma_start_transpose(
                            ybuf[:, k, mm * 128:(mm + 1) * 128],
                            estore[:, q * 4 + mm, k * 128:(k + 1) * 128])
                for h in range(4):                    # channel quarters of 512
                    rh = rp.tile([128, 32, 512], bf, tag="rh")
                    for k in range(32):
                        nc.sync.dma_start(rh[:, k, :], rT[k][:, h * 512:(h + 1) * 512])
                    for mm in range(4):
                        u = ps.tile([128, 512], f32, tag="u")
                        for k in range(32):
                            nc.tensor.matmul(
                                u[:], ybuf[:, k, mm * 128:(mm + 1) * 128], rh[:, k, :],
                                start=(k == 0), stop=(k == 31))
                        o = wk.tile([128, 512], f32, tag="o")
                        nc.scalar.copy(o[:], u[:])
                        r0 = q * 512 + mm * 128
                        nc.sync.dma_start(uT[r0:r0 + 128, h * 512:(h + 1) * 512], o[:])
    nc.compile()
    return nc


def _get_nc():
    global _NC
    if _NC is None:
        _NC = _build_nc()
    return _NC


def _im2col3(x, pad):
    # x [C,H,W] -> [C*9, H*W] rows ordered (c,u,v), dilation=pad rate
    Cc, H, W = x.shape
    r = pad
    xp = np.pad(x, ((0, 0), (r, r), (r, r)))
    cols = np.empty((Cc, 3, 3, H, W), dtype=x.dtype)
    for u in range(3):
        for v in range(3):
            cols[:, u, v] = xp[:, u * r:u * r + H, v * r:v * r + W]
    return cols.reshape(Cc * 9, H * W)


def _raw_patches(x1s):
    # x1s [C,128,128] -> R [L, C*16] rows l=(i,j) row-major, cols (c,di,dj)
    xp = np.pad(x1s, ((0, 0), (1, 1), (1, 1)))
    R = np.empty((C, 4, 4, H2, H2), dtype=x1s.dtype)
    for di in range(4):
        for dj in range(4):
            R[:, di, dj] = xp[:, di:di + 128:2, dj:dj + 128:2]
    return R.transpose(3, 4, 0, 1, 2).reshape(L, C * 16)


def _col2im(Ut):
    # Ut [L, C*16] -> y [C,128,128] scatter-add, h=2i+di-1
    blk = Ut.reshape(H2, H2, C, 4, 4).transpose(2, 3, 4, 0, 1)
    acc = np.zeros((C, 130, 130), dtype=np.float32)
    for di in range(4):
        for dj in range(4):
            acc[:, di:di + 128:2, dj:dj + 128:2] += blk[:, di, dj]
    return (acc[:, 1:129, 1:129] / 4.0).astype(np.float32)


def kernel(x1, x2, mask, fw0, fb0, fw1, fb1, fw2, fb2, fw3, fb3):
    from concourse.bass_utils import run_bass_kernel_spmd

    x1 = np.asarray(x1, np.float32)
    x2 = np.asarray(x2, np.float32)
    mask = np.asarray(mask, np.float32)
    fws = [np.asarray(f, np.float32) for f in (fw0, fw1, fw2, fw3)]
    fbs = [np.asarray(f, np.float32) for f in (fb0, fb1, fb2, fb3)]

    nc = _get_nc()
    in_maps = []
    rawRs, mms = [], []
    for s in range(B):
        cols = _im2col3(x2[s], 1)                      # [1152, 4096]
        norms = np.sqrt((cols * cols).sum(0, dtype=np.float32))
        mp = np.pad(mask[s, 0], 1)
        psum = np.zeros((H2, H2), np.float32)
        for u in range(3):
            for v in range(3):
                psum += mp[u:u + H2, v:v + H2]
        mm = (psum.reshape(-1) == 0.0).astype(np.float32)   # [L] 1=valid
        aT = (cols * (SCALE * mm / np.maximum(norms, EPS_NORM))[None, :])
        aT = aT.astype(BF16).reshape(9, 128, L)
        rawR = _raw_patches(x1[s])                     # [L, 2048]
        rT = (rawR * mm[:, None]).astype(BF16).reshape(32, 128, RDIM)
        rawRs.append(rawR)
        mms.append(mm)
        for q in range(4):
            xTq = cols[:, q * POSL:(q + 1) * POSL].astype(BF16).reshape(9, 128, POSL)
            in_maps.append({"xT": xTq, "aT": aT, "rT": rT})

    res = run_bass_kernel_spmd(nc, in_maps, core_ids=list(range(8)))

    out = np.empty((B, 64, H1, H1), np.float32)
    for s in range(B):
        Ut = np.concatenate([res.results[s * 4 + q]["uT"] for q in range(4)], 0)
        # clip correction: masked patches contribute exactly EPS_CLAMP * raw patch
        corr = EPS_CLAMP * rawRs[s][mms[s] == 0.0].sum(0, dtype=np.float64)
        y = _col2im(Ut + corr[None, :].astype(np.float32))
        for ri, r in enumerate((1, 2, 4, 8)):
            colsY = _im2col3(y, r)                     # [1152, 16384]
            o = fws[ri].reshape(16, KDIM) @ colsY + fbs[ri][:, None]
            out[s, ri * 16:(ri + 1) * 16] = np.maximum(o, 0.0).reshape(16, H1, H1)
    return out



# revision 7
# speedup vs baseline: 4.0391x; 4.0391x over previous
"""Distributed Trainium2 Bass kernel for AtnConv (contextual-attention conv).

Sharding: 8 cores = batch(2) x position-blocks(4). Each core owns 1024 of the
4096 output positions of one sample and attends over ALL L=4096 patches, so the
softmax over L is core-local.

Tunnel-transfer-minimal design: only raw images ship to the device —
  xq   [128, 18, 66]  bf16  x2 canvas rows for this core's position quarter
  x2p  [128, 66, 66]  bf16  padded x2 (3x3 im2col built on device via DMA)
  x1p  [128,130,130]  bf16  padded x1 (4x4/stride2 patches built on device)
  svec [1, 4096]      f32   per-patch score scale = SCALE*mm/max(norm,eps)
  mmT  [128, 32]      f32   valid-patch mask, [l%128, l//128] layout
and only a col2im'd strip returns:
  ys   [128, 34, 130] bf16  this quarter's conv_transpose partial (x4, pre-crop)

Device: scores S^T[pos,L] = cols_q^T @ cols (bf16 matmul, f32 psum), scaled by
svec in f32, masked softmax over L (block max + deferred rescale), then
U[ch,pos] = R^T @ Y^T per 128-patch chunk with R chunks built by DMA-transpose
from x1p, accumulated straight into the strided col2im strip.
Host: norms/mask prep, strip stitching, eps-clip correction, final 4 convs.
"""

import numpy as np
import ml_dtypes

B, C, H1, H2 = 2, 128, 128, 64
L = H2 * H2            # 4096 patches
POSL = 1024            # positions per core
KDIM = 1152            # 128*3*3 contraction for scores
SCALE = 10.0
EPS_NORM = 1e-4
EPS_CLAMP = 1e-8
BF16 = ml_dtypes.bfloat16

_NC = None


def _build_nc():
    import concourse.bass as bass
    import concourse.bacc as bacc
    import concourse.mybir as mybir
    from concourse import tile

    bf = mybir.dt.bfloat16
    f32 = mybir.dt.float32
    Exp = mybir.ActivationFunctionType.Exp
    X = mybir.AxisListType.X

    nc = bacc.Bacc(None, target_bir_lowering=False)
    xq = nc.declare_dram_parameter("xq", [128, 18, 66], bf, isOutput=False)
    x2p = nc.declare_dram_parameter("x2p", [128, 66, 66], bf, isOutput=False)
    x1p = nc.declare_dram_parameter("x1p", [128, 2, 2, 65, 65], bf, isOutput=False)
    svec = nc.declare_dram_parameter("svec", [1, L], f32, isOutput=False)
    mmT = nc.declare_dram_parameter("mmT", [128, 32], f32, isOutput=False)
    ys = nc.declare_dram_parameter("ys", [128, 34, 130], bf, isOutput=True)

    with tile.TileContext(nc) as tc:
        with (
            tc.tile_pool(name="big", bufs=1) as big,
            tc.tile_pool(name="st", bufs=1) as st,
            tc.tile_pool(name="wk", bufs=2) as wk,
            tc.tile_pool(name="ps", bufs=2, space=bass.MemorySpace.PSUM) as ps,
        ):
            estore = big.tile([128, 8, L], bf)        # 64 KiB/part
            xt = big.tile([128, 9, POSL], bf)         # 18 KiB
            srep = big.tile([128, L], f32)            # 16 KiB
            mmt = big.tile([128, 32], f32)
            ystrip = big.tile([128, 34, 130], f32)    # 17.7 KiB
            ysb = big.tile([128, 34, 130], bf)        # 8.8 KiB
            nbmaxs = st.tile([128, 8, 8], f32)
            rss = st.tile([128, 8, 8], f32)
            svs = st.tile([1, L], f32)

            nc.sync.dma_start(mmt[:], mmT[:])
            nc.sync.dma_start(svs[:], svec[:])
            nc.gpsimd.partition_broadcast(srep[:], svs[:], channels=128)
            for k in range(9):
                u, v = k // 3, k % 3
                nc.sync.dma_start(xt[:, k, :], xq[:, u:u + 16, v:v + 64])

            # ---- scores: z = cols_q^T @ cols, scale in f32, block-local exp ----
            for n in range(8):                        # L blocks of 512
                a_n = wk.tile([128, 9, 512], bf, tag="a_n")
                for k in range(9):
                    u, v = k // 3, k % 3
                    nc.sync.dma_start(a_n[:, k, :],
                                      x2p[:, 8 * n + u:8 * n + u + 8, v:v + 64])
                for m in range(8):                    # pos tiles of 128
                    z = ps.tile([128, 512], f32, tag="z")
                    for k in range(9):
                        nc.tensor.matmul(
                            z[:], xt[:, k, m * 128:(m + 1) * 128], a_n[:, k, :],
                            start=(k == 0), stop=(k == 8))
                    zs = wk.tile([128, 512], f32, tag="zs")
                    nc.vector.tensor_mul(zs[:], z[:], srep[:, n * 512:(n + 1) * 512])
                    # nbmaxs holds NEGATED block maxes (reduce negate=True)
                    nc.vector.reduce_max(nbmaxs[:, m, n:n + 1], zs[:], axis=X,
                                         negate=True)
                    ef = wk.tile([128, 512], f32, tag="ef")
                    nc.scalar.activation(ef[:], zs[:], Exp,
                                         bias=nbmaxs[:, m, n:n + 1], scale=1.0)
                    nc.vector.reduce_sum(rss[:, m, n:n + 1], ef[:], axis=X)
                    nc.vector.tensor_copy(estore[:, m, n * 512:(n + 1) * 512], ef[:])

            # ---- softmax finalize: rescale each block by exp(bmax-gmax)/Z ----
            for m in range(8):
                ngm = wk.tile([128, 1], f32, tag="ngm")
                nc.vector.tensor_reduce(ngm[:], nbmaxs[:, m, :], axis=X,
                                        op=mybir.AluOpType.min)
                al = wk.tile([128, 8], f32, tag="al")
                nc.scalar.activation(al[:], nbmaxs[:, m, :], Exp, bias=ngm[:],
                                     scale=-1.0)
                pr = wk.tile([128, 8], f32, tag="pr")
                nc.vector.tensor_mul(pr[:], al[:], rss[:, m, :])
                sm = wk.tile([128, 1], f32, tag="sm")
                nc.vector.reduce_sum(sm[:], pr[:], axis=X)
                rc = wk.tile([128, 1], f32, tag="rc")
                nc.vector.reciprocal(rc[:], sm[:])
                be = wk.tile([128, 8], f32, tag="be")
                nc.vector.tensor_scalar_mul(be[:], al[:], rc[:])
                for n in range(8):
                    nc.vector.tensor_scalar_mul(
                        estore[:, m, n * 512:(n + 1) * 512],
                        estore[:, m, n * 512:(n + 1) * 512], be[:, n:n + 1])

            # ---- per 128-patch chunk: Y^T transpose, R build, U, col2im ----
            nc.vector.memset(ystrip[:], 0.0)
            for lc in range(32):
                yb = wk.tile([128, 1024], bf, tag="yb")
                for m in range(8):
                    nc.sync.dma_start_transpose(
                        yb[:, m * 128:(m + 1) * 128],
                        estore[:, m, lc * 128:(lc + 1) * 128])
                nc.vector.tensor_scalar_mul(yb[:], yb[:], mmt[:, lc:lc + 1])
                rh = wk.tile([128, 16, 128], bf, tag="rh")
                for b in range(16):
                    di, dj = b // 4, b % 4
                    pst = wk.tile([128, 2, 64], bf, tag="pst")
                    r2 = 2 * lc + di // 2
                    nc.sync.dma_start(
                        pst[:],
                        x1p[:, di % 2, dj % 2, r2:r2 + 2, dj // 2:dj // 2 + 64])
                    nc.sync.dma_start_transpose(rh[:, b, :], pst[:])
                for b in range(16):
                    di, dj = b // 4, b % 4
                    for h in range(2):
                        pu = ps.tile([128, 8, 64], f32, tag="pu")
                        nc.tensor.matmul(pu[:], rh[:, b, :],
                                         yb[:, h * 512:(h + 1) * 512],
                                         start=True, stop=True)
                        dst = ystrip[:, di + 16 * h:di + 16 * h + 15:2,
                                     dj:dj + 127:2]
                        nc.vector.tensor_add(dst, dst, pu[:])

            nc.vector.tensor_copy(ysb[:], ystrip[:])
            nc.sync.dma_start(ys[:], ysb[:])
    nc.compile()
    return nc


def _get_nc():
    global _NC
    if _NC is None:
        _NC = _build_nc()
    return _NC


def _im2col3(x, pad):
    # x: [C,H,W] -> [C*9, H*W] rows ordered (c,u,v), dilation=pad rate
    Cc, H, W = x.shape
    r = pad
    xp = np.pad(x, ((0, 0), (r, r), (r, r)))
    cols = np.empty((Cc, 3, 3, H, W), dtype=x.dtype)
    for u in range(3):
        for v in range(3):
            cols[:, u, v] = xp[:, u * r:u * r + H, v * r:v * r + W]
    return cols.reshape(Cc * 9, H * W)


def kernel(x1, x2, mask, fw0, fb0, fw1, fb1, fw2, fb2, fw3, fb3):
    from concourse.bass_utils import run_bass_kernel_spmd

    x1 = np.asarray(x1, np.float32)
    x2 = np.asarray(x2, np.float32)
    mask = np.asarray(mask, np.float32)
    fws = [np.asarray(f, np.float32) for f in (fw0, fw1, fw2, fw3)]
    fbs = [np.asarray(f, np.float32) for f in (fb0, fb1, fb2, fb3)]

    nc = _get_nc()
    in_maps = []
    corrs = []
    for s in range(B):
        x2c = np.zeros((C, 66, 66), np.float32)
        x2c[:, 1:65, 1:65] = x2[s]
        x1c = np.zeros((C, 130, 130), np.float32)
        x1c[:, 1:129, 1:129] = x1[s]
        # mm: 1 where the 3x3 mask patch is all zero (fully valid)
        mp = np.pad(mask[s, 0], 1)
        msum = np.zeros((H2, H2), np.float32)
        for u in range(3):
            for v in range(3):
                msum += mp[u:u + H2, v:v + H2]
        mm = (msum == 0.0).astype(np.float32)          # [64, 64]
        # patch norms of x2 (3x3, pad 1), f32 to match reference
        p2 = (x2[s] * x2[s]).sum(0)
        p2p = np.pad(p2, 1)
        n2 = np.zeros((H2, H2), np.float32)
        for u in range(3):
            for v in range(3):
                n2 += p2p[u:u + H2, v:v + H2]
        norms = np.sqrt(n2)
        sv = (SCALE * mm / np.maximum(norms, EPS_NORM)).reshape(1, L)
        mmTa = np.ascontiguousarray(mm.reshape(32, 128).T.astype(np.float32))
        x2cb = x2c.astype(BF16)
        x1q = np.empty((C, 2, 2, 65, 65), BF16)
        for pr in range(2):
            for pc in range(2):
                x1q[:, pr, pc] = x1c[:, pr::2, pc::2]
        sv = np.ascontiguousarray(sv.astype(np.float32))
        for q in range(4):
            xqa = np.ascontiguousarray(x2cb[:, 16 * q:16 * q + 18, :])
            in_maps.append({"xq": xqa, "x2p": x2cb, "x1p": x1q,
                            "svec": sv, "mmT": mmTa})
        # eps-clip correction: masked patches contribute EPS_CLAMP * raw patch
        mask0 = mm.reshape(L) == 0.0
        corr = np.zeros((16, C), np.float32)
        for di in range(4):
            for dj in range(4):
                pl = x1c[:, di:di + 127:2, dj:dj + 127:2].reshape(C, L)
                corr[di * 4 + dj] = EPS_CLAMP * pl[:, mask0].sum(1)
        corrs.append(corr)

    res = run_bass_kernel_spmd(nc, in_maps, core_ids=list(range(8)))

    out = np.empty((B, 64, H1, H1), np.float32)
    for s in range(B):
        acc = np.zeros((C, 130, 130), np.float32)
        for q in range(4):
            strip = np.asarray(res.results[s * 4 + q]["ys"], dtype=np.float32)
            acc[:, 32 * q:32 * q + 34, :] += strip
        corr = corrs[s]
        for di in range(4):
            for dj in range(4):
                acc[:, di:di + 127:2, dj:dj + 127:2] += \
                    corr[di * 4 + dj][:, None, None]
        y = acc[:, 1:129, 1:129] / 4.0
        for ri, r in enumerate((1, 2, 4, 8)):
            colsY = _im2col3(y, r)                     # [1152, 16384]
            o = fws[ri].reshape(16, KDIM) @ colsY + fbs[ri][:, None]
            out[s, ri * 16:(ri + 1) * 16] = np.maximum(o, 0.0).reshape(16, H1, H1)
    return out


# revision 9
# speedup vs baseline: 6.2108x; 1.5377x over previous
"""Distributed Trainium2 Bass kernel for AtnConv (contextual-attention conv).

Sharding: 8 cores = batch(2) x position-blocks(4). Each core owns 1024 of the
4096 output positions of one sample and attends over ALL L=4096 patches, so the
softmax over L is core-local.

Tunnel-transfer-minimal design: only raw images ship to the device —
  xq   [128, 18, 66]  bf16  x2 canvas rows for this core's position quarter
  x2p  [128, 66, 66]  bf16  padded x2 (3x3 im2col built on device via DMA)
  x1p  [128,130,130]  bf16  padded x1 (4x4/stride2 patches built on device)
  svec [1, 4096]      f32   per-patch score scale = SCALE*mm/max(norm,eps)
  mmT  [128, 32]      f32   valid-patch mask, [l%128, l//128] layout
and only a col2im'd strip returns:
  ys   [128, 34, 130] bf16  this quarter's conv_transpose partial (x4, pre-crop)

Device: scores S^T[pos,L] = cols_q^T @ cols (bf16 matmul, f32 psum), scaled by
svec in f32, masked softmax over L (block max + deferred rescale), then
U[ch,pos] = R^T @ Y^T per 128-patch chunk with R chunks built by DMA-transpose
from x1p, accumulated straight into the strided col2im strip.
Host: norms/mask prep, strip stitching, eps-clip correction, final 4 convs.
"""

import numpy as np
import ml_dtypes

B, C, H1, H2 = 2, 128, 128, 64
L = H2 * H2            # 4096 patches
POSL = 1024            # positions per core
KDIM = 1152            # 128*3*3 contraction for scores
SCALE = 10.0
EPS_NORM = 1e-4
EPS_CLAMP = 1e-8
BF16 = ml_dtypes.bfloat16

_NC = None


def _build_nc():
    import concourse.bass as bass
    import concourse.bacc as bacc
    import concourse.mybir as mybir
    from concourse import tile

    bf = mybir.dt.bfloat16
    f32 = mybir.dt.float32
    Exp = mybir.ActivationFunctionType.Exp
    X = mybir.AxisListType.X

    nc = bacc.Bacc(None, target_bir_lowering=False)
    xq = nc.declare_dram_parameter("xq", [128, 18, 66], bf, isOutput=False)
    x2part = nc.declare_dram_parameter("x2part", [32, 66, 66], bf, isOutput=False)
    x1part = nc.declare_dram_parameter("x1part", [32, 2, 2, 65, 65], bf,
                                       isOutput=False)
    svec = nc.declare_dram_parameter("svec", [1, L], f32, isOutput=False)
    mmT = nc.declare_dram_parameter("mmT", [128, 32], f32, isOutput=False)
    ys = nc.declare_dram_parameter("ys", [128, 34, 130], bf, isOutput=True)

    GROUPS = [[0, 1, 2, 3], [4, 5, 6, 7]]
    with tile.TileContext(nc) as tc:
        with (
            tc.tile_pool(name="dram", bufs=1, space="DRAM") as dram,
            tc.tile_pool(name="big", bufs=1) as big,
            tc.tile_pool(name="st", bufs=1) as st,
            tc.tile_pool(name="wk", bufs=2) as wk,
            tc.tile_pool(name="ps", bufs=2, space=bass.MemorySpace.PSUM) as ps,
        ):
            # gather the 4x-replicated sample tensors from quarter slices
            ib2 = dram.tile([32, 66, 66], bf)
            x2p = dram.tile([128, 66, 66], bf)
            ib1 = dram.tile([32, 2, 2, 65, 65], bf)
            x1p = dram.tile([128, 2, 2, 65, 65], bf)
            nc.gpsimd.dma_start(ib2[:], x2part[:])
            nc.gpsimd.collective_compute(
                "AllGather", mybir.AluOpType.bypass, replica_groups=GROUPS,
                ins=[ib2.opt()], outs=[x2p.opt()])
            nc.gpsimd.dma_start(ib1[:], x1part[:])
            nc.gpsimd.collective_compute(
                "AllGather", mybir.AluOpType.bypass, replica_groups=GROUPS,
                ins=[ib1.opt()], outs=[x1p.opt()])
            estore = big.tile([128, 8, L], bf)        # 64 KiB/part
            xt = big.tile([128, 9, POSL], bf)         # 18 KiB
            srep = big.tile([128, L], f32)            # 16 KiB
            mmt = big.tile([128, 32], f32)
            ystrip = big.tile([128, 34, 130], f32)    # 17.7 KiB
            ysb = big.tile([128, 34, 130], bf)        # 8.8 KiB
            nbmaxs = st.tile([128, 8, 8], f32)
            rss = st.tile([128, 8, 8], f32)
            svs = st.tile([1, L], f32)

            nc.sync.dma_start(mmt[:], mmT[:])
            nc.sync.dma_start(svs[:], svec[:])
            nc.gpsimd.partition_broadcast(srep[:], svs[:], channels=128)
            for k in range(9):
                u, v = k // 3, k % 3
                nc.sync.dma_start(xt[:, k, :], xq[:, u:u + 16, v:v + 64])

            # ---- scores: z = cols_q^T @ cols, scale in f32, block-local exp ----
            for n in range(8):                        # L blocks of 512
                a_n = wk.tile([128, 9, 512], bf, tag="a_n")
                for k in range(9):
                    u, v = k // 3, k % 3
                    nc.sync.dma_start(a_n[:, k, :],
                                      x2p[:, 8 * n + u:8 * n + u + 8, v:v + 64])
                for m in range(8):                    # pos tiles of 128
                    z = ps.tile([128, 512], f32, tag="z")
                    for k in range(9):
                        nc.tensor.matmul(
                            z[:], xt[:, k, m * 128:(m + 1) * 128], a_n[:, k, :],
                            start=(k == 0), stop=(k == 8))
                    zs = wk.tile([128, 512], f32, tag="zs")
                    nc.vector.tensor_mul(zs[:], z[:], srep[:, n * 512:(n + 1) * 512])
                    # nbmaxs holds NEGATED block maxes (reduce negate=True)
                    nc.vector.reduce_max(nbmaxs[:, m, n:n + 1], zs[:], axis=X,
                                         negate=True)
                    ef = wk.tile([128, 512], f32, tag="ef")
                    nc.scalar.activation(ef[:], zs[:], Exp,
                                         bias=nbmaxs[:, m, n:n + 1], scale=1.0)
                    nc.vector.reduce_sum(rss[:, m, n:n + 1], ef[:], axis=X)
                    nc.vector.tensor_copy(estore[:, m, n * 512:(n + 1) * 512], ef[:])

            # ---- softmax finalize: rescale each block by exp(bmax-gmax)/Z ----
            for m in range(8):
                ngm = wk.tile([128, 1], f32, tag="ngm")
                nc.vector.tensor_reduce(ngm[:], nbmaxs[:, m, :], axis=X,
                                        op=mybir.AluOpType.min)
                al = wk.tile([128, 8], f32, tag="al")
                nc.scalar.activation(al[:], nbmaxs[:, m, :], Exp, bias=ngm[:],
                                     scale=-1.0)
                pr = wk.tile([128, 8], f32, tag="pr")
                nc.vector.tensor_mul(pr[:], al[:], rss[:, m, :])
                sm = wk.tile([128, 1], f32, tag="sm")
                nc.vector.reduce_sum(sm[:], pr[:], axis=X)
                rc = wk.tile([128, 1], f32, tag="rc")
                nc.vector.reciprocal(rc[:], sm[:])
                be = wk.tile([128, 8], f32, tag="be")
                nc.vector.tensor_scalar_mul(be[:], al[:], rc[:])
                for n in range(8):
                    nc.vector.tensor_scalar_mul(
                        estore[:, m, n * 512:(n + 1) * 512],
                        estore[:, m, n * 512:(n + 1) * 512], be[:, n:n + 1])

            # ---- per 128-patch chunk: Y^T transpose, R build, U, col2im ----
            nc.vector.memset(ystrip[:], 0.0)
            for lc in range(32):
                yb = wk.tile([128, 1024], bf, tag="yb")
                for m in range(8):
                    nc.sync.dma_start_transpose(
                        yb[:, m * 128:(m + 1) * 128],
                        estore[:, m, lc * 128:(lc + 1) * 128])
                nc.vector.tensor_scalar_mul(yb[:], yb[:], mmt[:, lc:lc + 1])
                rh = wk.tile([128, 16, 128], bf, tag="rh")
                for b in range(16):
                    di, dj = b // 4, b % 4
                    pst = wk.tile([128, 2, 64], bf, tag="pst")
                    r2 = 2 * lc + di // 2
                    nc.sync.dma_start(
                        pst[:],
                        x1p[:, di % 2, dj % 2, r2:r2 + 2, dj // 2:dj // 2 + 64])
                    nc.sync.dma_start_transpose(rh[:, b, :], pst[:])
                for b in range(16):
                    di, dj = b // 4, b % 4
                    for h in range(2):
                        pu = ps.tile([128, 8, 64], f32, tag="pu")
                        nc.tensor.matmul(pu[:], rh[:, b, :],
                                         yb[:, h * 512:(h + 1) * 512],
                                         start=True, stop=True)
                        dst = ystrip[:, di + 16 * h:di + 16 * h + 15:2,
                                     dj:dj + 127:2]
                        nc.vector.tensor_add(dst, dst, pu[:])

            nc.vector.tensor_copy(ysb[:], ystrip[:])
            nc.sync.dma_start(ys[:], ysb[:])
    nc.compile()
    return nc


def _get_nc():
    global _NC
    if _NC is None:
        _NC = _build_nc()
    return _NC


def _im2col3(x, pad):
    # x: [C,H,W] -> [C*9, H*W] rows ordered (c,u,v), dilation=pad rate
    Cc, H, W = x.shape
    r = pad
    xp = np.pad(x, ((0, 0), (r, r), (r, r)))
    cols = np.empty((Cc, 3, 3, H, W), dtype=x.dtype)
    for u in range(3):
        for v in range(3):
            cols[:, u, v] = xp[:, u * r:u * r + H, v * r:v * r + W]
    return cols.reshape(Cc * 9, H * W)


def kernel(x1, x2, mask, fw0, fb0, fw1, fb1, fw2, fb2, fw3, fb3):
    from concourse.bass_utils import run_bass_kernel_spmd

    x1 = np.asarray(x1, np.float32)
    x2 = np.asarray(x2, np.float32)
    mask = np.asarray(mask, np.float32)
    fws = [np.asarray(f, np.float32) for f in (fw0, fw1, fw2, fw3)]
    fbs = [np.asarray(f, np.float32) for f in (fb0, fb1, fb2, fb3)]

    nc = _get_nc()
    in_maps = []
    corrs = []
    for s in range(B):
        x2c = np.zeros((C, 66, 66), np.float32)
        x2c[:, 1:65, 1:65] = x2[s]
        x1c = np.zeros((C, 130, 130), np.float32)
        x1c[:, 1:129, 1:129] = x1[s]
        # mm: 1 where the 3x3 mask patch is all zero (fully valid)
        mp = np.pad(mask[s, 0], 1)
        msum = np.zeros((H2, H2), np.float32)
        for u in range(3):
            for v in range(3):
                msum += mp[u:u + H2, v:v + H2]
        mm = (msum == 0.0).astype(np.float32)          # [64, 64]
        # patch norms of x2 (3x3, pad 1), f32 to match reference
        p2 = (x2[s] * x2[s]).sum(0)
        p2p = np.pad(p2, 1)
        n2 = np.zeros((H2, H2), np.float32)
        for u in range(3):
            for v in range(3):
                n2 += p2p[u:u + H2, v:v + H2]
        norms = np.sqrt(n2)
        sv = (SCALE * mm / np.maximum(norms, EPS_NORM)).reshape(1, L)
        mmTa = np.ascontiguousarray(mm.reshape(32, 128).T.astype(np.float32))
        x2cb = x2c.astype(BF16)
        x1q = np.empty((C, 2, 2, 65, 65), BF16)
        for pr in range(2):
            for pc in range(2):
                x1q[:, pr, pc] = x1c[:, pr::2, pc::2]
        sv = np.ascontiguousarray(sv.astype(np.float32))
        for q in range(4):
            xqa = np.ascontiguousarray(x2cb[:, 16 * q:16 * q + 18, :])
            in_maps.append({
                "xq": xqa,
                "x2part": np.ascontiguousarray(x2cb[32 * q:32 * q + 32]),
                "x1part": np.ascontiguousarray(x1q[32 * q:32 * q + 32]),
                "svec": sv, "mmT": mmTa})
        # eps-clip correction: masked patches contribute EPS_CLAMP * raw patch
        mask0 = mm.reshape(L) == 0.0
        corr = np.zeros((16, C), np.float32)
        for di in range(4):
            for dj in range(4):
                pl = x1c[:, di:di + 127:2, dj:dj + 127:2].reshape(C, L)
                corr[di * 4 + dj] = EPS_CLAMP * pl[:, mask0].sum(1)
        corrs.append(corr)

    res = run_bass_kernel_spmd(nc, in_maps, core_ids=list(range(8)))

    out = np.empty((B, 64, H1, H1), np.float32)
    for s in range(B):
        acc = np.zeros((C, 130, 130), np.float32)
        for q in range(4):
            strip = np.asarray(res.results[s * 4 + q]["ys"], dtype=np.float32)
            acc[:, 32 * q:32 * q + 34, :] += strip
        corr = corrs[s]
        for di in range(4):
            for dj in range(4):
                acc[:, di:di + 127:2, dj:dj + 127:2] += \
                    corr[di * 4 + dj][:, None, None]
        y = acc[:, 1:129, 1:129] / 4.0
        for ri, r in enumerate((1, 2, 4, 8)):
            colsY = _im2col3(y, r)                     # [1152, 16384]
            o = fws[ri].reshape(16, KDIM) @ colsY + fbs[ri][:, None]
            out[s, ri * 16:(ri + 1) * 16] = np.maximum(o, 0.0).reshape(16, H1, H1)
    return out
